# revision 11
# baseline (speedup 1.0000x reference)
"""GCN layer kernel for 8 trn2 NeuronCores.

Math:  out = D (A + I) D feature W^T + b      (D = diag(hat_d))
Rewritten with g = (hat_d * feature) @ W^T  (the linear commutes with the
row-scaling and the SpMM):
    out = hat_d * (A @ g) + hat_d * g + b
       = hat_d * A @ g + E2,   E2 = hat_d * corr + b + hat_d * g_own
(corr is the fp8 mean-subtraction compensation, below; E2 is folded into
one SBUF buffer during the main matmul so the epilogue is two
tensor-tensor ops per output block, split across DVE and GpSimd.)

Sharding: A row-sharded across 8 cores (2048 rows each). Each core
computes full g locally from a replicated feature^T (measured: an 8MB
AllGather costs ~110us on this fabric — far more than the 55us of
replicated phase-1 PE work it would save, so no collectives).

Device layout: the big matmul is computed transposed,
out_sh^T[o, m] = sum_j g[j, o] * A_sh^T[j, m], so g tiles are the
stationary operand and the A shard (pre-transposed on the host — lhsT
layout prep for the systolic array) is the moving operand in natural
layout. The host applies an "own rows first" node permutation to the j
axis of A^T / feature^T / hat_d so the same SPMD program works on every
core (own-shard g tiles are always j = 0..15). Each (k, h) weight load
covers four 512-col matmuls so LDWEIGHTS stays hidden in the PE reorder
window (m-chunking the accumulators was measured ~17us slower from the
doubled LDW stream).

Phase-1 is DMA-bandwidth critical (~235 GB/s of feature stream against
a ~250 GB/s effective per-core HBM cap): features stream in 1MB blocks
over three rotating queues, and the A stream is emitted behind them on
sync+scalar so its prefetch cannot steal bandwidth during phase-1
(A isn't needed until ~100us in; 1MB feature granularity keeps any PE
stall under the 3.4us HAM window that would halve the clock).

A is stored fp8 e3m4 of (A - 0.5): the mean-subtraction centers the
uniform[0,1) entries so the fp8 relative error applies to half the
magnitude, and the exact rank-1 term corr = 0.5 * colsum(g) is added
back via E2. colsum is accumulated off the critical path on DVE (one
fold per feature block, pair-folds for the last block, then a tree) and
partition-reduced with two XBAR DMA transposes + free-axis reduces on
the lightly-loaded sync queue — no PE matmuls, no PSUM, so the ps1 ->
main-loop PSUM pool handoff never waits on it. fp8 halves A's HBM
traffic vs fp16; measured end-to-end relative error ~8e-3. g / feature
/ W stay fp16 (their error propagates ~1:1 to the output). The mixed
fp16(stationary) x fp8(moving) matmul runs at the full 16-bit PE rate
(measured: 216ns issue gap per 512-col matmul).

The output is written fp16 (host upcasts).
"""

import os

import ml_dtypes
import numpy as np

import concourse.mybir as mybir
import concourse.tile as tile
from concourse import bacc
from concourse.bass_utils import run_bass_kernel_spmd
from concourse.masks import make_identity

N = 16384
F = 512  # in features
O = 256  # out features
NCORES = 8
SH = N // NCORES  # 2048 rows per core
JT = N // 128  # 128 node tiles
NB = 1024  # phase-1 node-block width (per feature slab)
NBLK = N // NB  # 16 feature blocks
TPB = NB // 128  # 8 node tiles per block
KB = 4  # node tiles per A DMA block (1MB transfers)

F32 = mybir.dt.float32
F16 = mybir.dt.float16
F8 = mybir.dt.float8e3  # e3m4

_CACHE = {}


def build_program():
    nc = bacc.Bacc("TRN2", target_bir_lowering=False, debug=False,
                   num_devices=NCORES, dynamic_dma_scratch_size=8192)

    at = nc.dram_tensor("at", [JT // KB, 128, KB * SH], F8,
                        kind="ExternalInput").ap()
    ft = nc.dram_tensor("ft", [F, N], F16, kind="ExternalInput").ap()
    hdt = nc.dram_tensor("hdt", [128, JT], F32, kind="ExternalInput").ap()
    hdo = nc.dram_tensor("hdo", [1, SH], F16, kind="ExternalInput").ap()
    wt = nc.dram_tensor("wt", [F, O], F16, kind="ExternalInput").ap()
    bvec = nc.dram_tensor("bvec", [O, 1], F32, kind="ExternalInput").ap()
    outT = nc.dram_tensor("outT", [O, SH], F16, kind="ExternalOutput").ap()

    add = mybir.AluOpType.add
    mult = mybir.AluOpType.mult

    with tile.TileContext(nc) as tc:
        with (
            tc.tile_pool(name="const", bufs=1) as constp,
            tc.tile_pool(name="gpool", bufs=1) as gp,
            tc.tile_pool(name="fslab", bufs=12) as fsp,
            tc.tile_pool(name="aslab", bufs=3) as asp,
            tc.tile_pool(name="tout", bufs=4) as wp,
            tc.tile_pool(name="scr", bufs=2) as scp,
        ):
            qs = [nc.sync, nc.scalar]
            qf = [nc.sync, nc.scalar, nc.gpsimd]

            # Small consts first: the first matmul needs wt, and these are
            # tiny (320KB total) compared to the feature stream behind them.
            wt_sb = constp.tile([128, 4 * O], F16, tag="wt")
            for fc in range(4):
                nc.scalar.dma_start(out=wt_sb[:, fc * O:(fc + 1) * O],
                                    in_=wt[fc * 128:(fc + 1) * 128, :])
            hdt_sb = constp.tile([128, JT], F32, tag="hdt")
            nc.sync.dma_start(out=hdt_sb[:], in_=hdt[:, :])

            ident = constp.tile([128, 128], F32, tag="ident")
            make_identity(nc, ident[:])

            # g for all nodes (fp16), node-tile j at columns [j*O, (j+1)*O)
            g_sb = gp.tile([128, JT * O], F16, tag="g")
            # E2 buffer: starts as e = (hat_d_own * g_own)^T (fp32), o-half h
            # at cols [h*SH, (h+1)*SH); later folded to corr*hd + b + e.
            e_sb = gp.tile([128, 2 * SH], F32, tag="e")
            # colsum accumulator (fp32), folded 2048 -> 256 at the end
            gsum = gp.tile([128, TPB * O], F32, tag="gsum")
            gsum16 = gp.tile([128, O], F16, tag="gsum16")
            # corr[o] = 0.5 * sum_j g[j, o], o-half h in column h
            corr_sb = constp.tile([128, 2], F32, tag="corr")

            # ---- phase 1: g = (hat_d * feature) @ W^T for all nodes ----
            # 1MB feature blocks over three rotating queues; the hat_d
            # row-scale alternates between DVE and ACT so neither engine
            # gates the PE stream.
            with tc.tile_pool(name="ps1", bufs=2, space="PSUM") as ps1:
                for jb in range(NBLK):
                    slabs = []
                    for fc in range(4):
                        s = fsp.tile([128, NB], F16, tag="fs",
                                     name=f"fs{jb}_{fc}")
                        qf[(4 * jb + fc) % 3].dma_start(
                            out=s[:],
                            in_=ft[fc * 128:(fc + 1) * 128,
                                   jb * NB:(jb + 1) * NB])
                        slabs.append(s)
                    for jj in range(TPB):
                        j = jb * TPB + jj
                        pfw = ps1.tile([128, O], F32, tag="fw", bufs=6)
                        for fc in range(4):
                            nc.tensor.matmul(
                                pfw[:],
                                lhsT=slabs[fc][:, jj * 128:(jj + 1) * 128],
                                rhs=wt_sb[:, fc * O:(fc + 1) * O],
                                start=(fc == 0), stop=(fc == 3))
                        gt = g_sb[:, j * O:(j + 1) * O]
                        if j % 2 == 0:
                            nc.vector.tensor_scalar_mul(
                                gt, pfw[:], hdt_sb[:, j:j + 1])
                        else:
                            nc.scalar.mul(
                                gt, pfw[:], hdt_sb[:, j:j + 1])
                        # last block: fold per tile-pair so only ~1 tile of
                        # colsum work remains after the last g tile.
                        if jb == NBLK - 1 and jj % 2 == 1:
                            q0 = jb * TPB * O + (jj - 1) * O
                            gs = slice((jj - 1) * O, (jj + 1) * O)
                            nc.vector.tensor_tensor(
                                gsum[:, gs], gsum[:, gs],
                                g_sb[:, q0:q0 + 2 * O], add)

                    if jb == 0:
                        nc.vector.tensor_copy(gsum[:], g_sb[:, :TPB * O])
                    elif jb < NBLK - 1:
                        blk = g_sb[:, jb * TPB * O:(jb + 1) * TPB * O]
                        nc.vector.tensor_tensor(gsum[:], gsum[:], blk, add)

                    if jb == 1:
                        # e = (hat_d_own * g_own)^T; own tiles are j = 0..15
                        # (blocks 0-1). Runs while later blocks stream in.
                        for jj in range(16):
                            for h in range(2):
                                sc = scp.tile([128, 128], F32, tag="sc")
                                nc.vector.tensor_scalar_mul(
                                    sc[:],
                                    g_sb[:, jj * O + h * 128:
                                         jj * O + (h + 1) * 128],
                                    hdt_sb[:, jj:jj + 1])
                                ptp = ps1.tile([128, 128], F32, tag="tp",
                                               bufs=2)
                                nc.tensor.transpose(ptp[:], sc[:], ident[:])
                                nc.vector.tensor_copy(
                                    e_sb[:, h * SH + jj * 128:
                                         h * SH + (jj + 1) * 128],
                                    ptp[:])

                # tree-fold gsum 2048 -> 256, then reduce across partitions
                # with an XBAR transpose (sync queue) + free-axis reduce: no
                # PE, no PSUM, so closing ps1 never waits on this.
                w = 4 * O
                while w >= O:
                    nc.vector.tensor_tensor(
                        gsum[:, :w], gsum[:, :w], gsum[:, w:2 * w], add)
                    w //= 2
                nc.vector.tensor_copy(gsum16[:], gsum[:, :O])
                for h in range(2):
                    gsT = scp.tile([128, 128], F16, tag="gsT")
                    nc.sync.dma_start(
                        out=gsT[:], in_=gsum16[:, h * 128:(h + 1) * 128],
                        transpose=True)
                    nc.vector.tensor_reduce(
                        corr_sb[:, h:h + 1], gsT[:],
                        axis=mybir.AxisListType.X, op=add)
                nc.vector.tensor_scalar_mul(corr_sb[:], corr_sb[:], 0.5)

            # epilogue-only constants: behind the feature stream, done long
            # before they are needed.
            b_sb = constp.tile([128, 2], F32, tag="b")
            for h in range(2):
                nc.scalar.dma_start(out=b_sb[:, h:h + 1],
                                    in_=bvec[h * 128:(h + 1) * 128, :])
            # hat_d of own rows broadcast across all partitions (free dim = m)
            hd_bc = constp.tile([128, SH], F16, tag="hdbc")
            nc.sync.dma_start(out=hd_bc[:],
                              in_=hdo[0:1, :].to_broadcast((128, SH)))

            # ---- main: acc[h] = (A_sh @ g)^T for o-half h ----
            with tc.tile_pool(name="ps2", bufs=1, space="PSUM") as psp:
                accs = [psp.tile([128, SH], F32, tag=f"acc{h}", name=f"acc{h}")
                        for h in range(2)]
                for k in range(JT):
                    if k % KB == 0:
                        sl4 = asp.tile([128, KB * SH], F8, tag="as")
                        qs[(k // KB) % 2].dma_start(
                            out=sl4[:], in_=at[k // KB, :, :])
                    q0 = (k % KB) * SH
                    for h in range(2):
                        lhs = g_sb[:, k * O + h * 128:k * O + (h + 1) * 128]
                        for mc in range(4):
                            nc.tensor.matmul(
                                accs[h][:, mc * 512:(mc + 1) * 512],
                                lhsT=lhs,
                                rhs=sl4[:, q0 + mc * 512:q0 + (mc + 1) * 512],
                                start=(k == 0), stop=(k == JT - 1))
                    if k == 8:
                        # fold E2 = corr*hd + b + e in place (free: DVE is
                        # idle during the main matmul, corr is ready)
                        for h in range(2):
                            hs = slice(h * SH, (h + 1) * SH)
                            nc.vector.scalar_tensor_tensor(
                                e_sb[:, hs], in0=hd_bc[:],
                                scalar=corr_sb[:, h:h + 1],
                                in1=e_sb[:, hs], op0=mult, op1=add)
                            nc.vector.tensor_scalar_add(
                                e_sb[:, hs], e_sb[:, hs], b_sb[:, h:h + 1])

                # ---- epilogue: out^T = acc * hd + E2 ----
                # TT1 on DVE; TT2 alternates DVE/GpSimd by block so the two
                # engines drain the tail in parallel.
                for h in range(2):
                    for c in range(4):
                        i = h * 4 + c
                        cs = slice(c * 512, (c + 1) * 512)
                        t = wp.tile([128, 512], F32, tag="t")
                        nc.vector.tensor_tensor(
                            t[:], accs[h][:, cs], hd_bc[:, cs], mult)
                        t16 = wp.tile([128, 512], F16, tag="t16")
                        eng2 = nc.gpsimd if i % 2 else nc.vector
                        eng2.tensor_tensor(
                            t16[:], t[:],
                            e_sb[:, h * SH + c * 512:h * SH + (c + 1) * 512],
                            add)
                        qs[(h + c) % 2].dma_start(
                            out=outT[h * 128:(h + 1) * 128, cs], in_=t16[:])

    nc.compile()
    return nc


def prep_inputs(A, hat_d, feature, W, b):
    """Per-core input maps. Host work is layout/dtype prep only: transpose,
    slice, concatenate (the own-rows-first node permutation on the j axis,
    plus the k-quad DMA blocking of A), and the fp32->fp8/fp16 dtype
    conversion for matmul operands (A is quantized as fp8(A - 0.5); the
    0.5 offset is restored on-device)."""
    A = np.ascontiguousarray(np.asarray(A, dtype=np.float32))
    hat_d = np.ascontiguousarray(np.asarray(hat_d, dtype=np.float32))
    feature = np.ascontiguousarray(np.asarray(feature, dtype=np.float32))
    W = np.asarray(W, dtype=np.float32)
    b = np.asarray(b, dtype=np.float32)

    featT = np.ascontiguousarray(feature.T.astype(np.float16))  # [F, N]
    wt = np.ascontiguousarray(W.T.astype(np.float16))  # [F, O]
    b2 = np.ascontiguousarray(b.reshape(O, 1))

    FP8 = ml_dtypes.float8_e3m4
    in_maps = []
    for c in range(NCORES):
        r0, r1 = c * SH, (c + 1) * SH
        rows = (A[r0:r1] - np.float32(0.5)).astype(FP8)  # [SH, N] fp8
        # A_sh^T with node (j) axis permuted own-rows-first
        at_c = np.empty((N, SH), dtype=FP8)
        at_c[:SH] = rows[:, r0:r1].T
        at_c[SH:SH + r0] = rows[:, :r0].T
        at_c[SH + r0:] = rows[:, r1:].T
        # blocked for the DMA stream: [k-quad][128][KB*SH]
        at_b = np.ascontiguousarray(
            at_c.reshape(JT // KB, KB, 128, SH)
            .transpose(0, 2, 1, 3)
            .reshape(JT // KB, 128, KB * SH))

        ft_c = np.empty((F, N), dtype=np.float16)
        ft_c[:, :SH] = featT[:, r0:r1]
        ft_c[:, SH:SH + r0] = featT[:, :r0]
        ft_c[:, SH + r0:] = featT[:, r1:]

        hd_c = np.concatenate([hat_d[r0:r1], hat_d[:r0], hat_d[r1:]])
        hdt_c = np.ascontiguousarray(hd_c.reshape(JT, 128).T)
        hdo_c = np.ascontiguousarray(
            hat_d[r0:r1].reshape(1, SH).astype(np.float16))

        in_maps.append({
            "at": at_b,
            "ft": ft_c,
            "hdt": hdt_c,
            "hdo": hdo_c,
            "wt": wt,
            "bvec": b2,
        })
    return in_maps


last_exec_time_ns = None
last_results = None


def kernel(A, hat_d, feature, W, b):
    global last_exec_time_ns, last_results
    if "nc" not in _CACHE:
        _CACHE["nc"] = build_program()
    nc = _CACHE["nc"]

    in_maps = prep_inputs(A, hat_d, feature, W, b)
    trace = bool(int(os.environ.get("KERNEL_TRACE", "0")))
    res = run_bass_kernel_spmd(nc, in_maps, list(range(NCORES)), trace=trace)
    last_exec_time_ns = res.exec_time_ns
    last_results = res

    out = np.empty((N, O), dtype=np.float32)
    for c in range(NCORES):
        out[c * SH:(c + 1) * SH] = res.results[c]["outT"].T.astype(np.float32)
    return out
